# revision 3
# baseline (speedup 1.0000x reference)
"""Trainium2 SPMD kernel for: y = BatchNorm1d(x @ sign(w).T + bias) * gamma + beta.

Sharding: data-parallel over the batch dim across 8 NeuronCores; the
weight is replicated.  BatchNorm batch statistics are produced with an
on-device AllReduce of per-shard (sum_y, sum_y2).

Layout: weight-stationary matmul producing y in [o, b] layout (output
features on partitions).  This makes the batch reductions free-dim
reductions (fused into the PSUM->SBUF copy via accum_out on DVE and the
Square pass on ACT), removes all stats matmuls from the tensor engine,
and turns the BN normalize into a single per-partition scale+bias op.

Math notes:
  - The linear bias cancels inside BatchNorm (y - mean), so it is never
    applied on device.
  - sign(w) in {-1,+1} is computed as (w >= 0) - 0.5 in {-0.5,+0.5}; the
    resulting global scale of 0.5 also cancels in BatchNorm except in the
    epsilon, which is compensated with eps/4.
  - Matmul runs in bf16 (weights +-0.5 exact; x rounding ~2e-3 rel err).
    x and w are cast to bf16 on the host (same numerics as a casting
    DMA, half the HBM traffic).  The output is written bf16 and upcast
    on the host (the values are bf16-rounded before the store either
    way, so this is numerically identical to a casting store DMA).
"""

import os
import sys

sys.path.insert(0, "/opt/trn_rl_repo")

import numpy as np

import concourse.bacc as bacc
import concourse.mybir as mybir
import concourse.tile as tile
from concourse import bass_utils

N_CORES = 8
B_TOT = 16384
D_IN = 2048
D_OUT = 1024
B_SH = B_TOT // N_CORES          # 2048 rows per core
KT = D_IN // 128                 # 16 contraction stripes
OC = D_OUT // 128                # 8 output chunks (partition dim of y)
BC = B_SH // 512                 # 4 batch chunks of 512 per core
BN_EPS = 1e-5

F32 = mybir.dt.float32
BF16 = mybir.dt.bfloat16

# oc passes after which a dummy warm-up all-reduce fires (keeps the CC
# firmware initialized/warm so the real stats AR at the end is cheap)
WARM_OCS = tuple(
    int(s) for s in os.environ.get("KERNEL_WARM_OCS", "0,4").split(",") if s != ""
)


def build_kernel():
    nc = bacc.Bacc("TRN2", target_bir_lowering=False, debug=False,
                   num_devices=N_CORES)

    # x^T per shard: [k, b], contiguous 4KB rows
    xt = nc.dram_tensor("xt", [D_IN, B_SH], BF16, kind="ExternalInput")
    # w^T: [k, o], contiguous 2KB rows
    wt = nc.dram_tensor("wt", [D_IN, D_OUT], BF16, kind="ExternalInput")
    # gamma/beta pre-laid-out as [128, OC] (partition p = o % 128, col oc)
    gamma = nc.dram_tensor("gamma", [128, OC], F32, kind="ExternalInput")
    beta = nc.dram_tensor("beta", [128, OC], F32, kind="ExternalInput")
    # output in device layout [oc*128 + p, b]; host transposes back
    out = nc.dram_tensor("out", [D_OUT, B_SH], BF16, kind="ExternalOutput")

    with tile.TileContext(nc) as tc:
        with tc.tile_pool(name="persist", bufs=1) as persist, \
             tc.tile_pool(name="work", bufs=2) as work_pool, \
             tc.tile_pool(name="stage", bufs=3) as stage_pool, \
             tc.tile_pool(name="psum", bufs=2, space="PSUM") as psum_pool, \
             tc.tile_pool(name="dram", bufs=1, space="DRAM") as dram:

            # ---- weights: load bf16 stripes, binarize to {-0.5,+0.5} ----
            # conversions split across DVE and ACT so stripe `it` is ready
            # before the tensor engine reaches it in the first oc pass.
            wbs = []
            for it in range(KT):
                wraw = work_pool.tile([128, D_OUT], BF16, name=f"wraw{it}",
                                      tag=f"wraw{it % 4}")
                eng = nc.scalar if it % 2 == 0 else nc.sync
                eng.dma_start(wraw[:], wt[it * 128:(it + 1) * 128, :])
                wb = persist.tile([128, D_OUT], BF16, name=f"wb{it}")
                if it % 2 == 0:
                    # (w >= 0) - 0.5 -> {+0.5, -0.5}
                    nc.vector.tensor_scalar(
                        out=wb[:], in0=wraw[:],
                        scalar1=0.0, scalar2=0.5,
                        op0=mybir.AluOpType.is_ge,
                        op1=mybir.AluOpType.subtract,
                    )
                else:
                    nc.gpsimd.tensor_scalar(
                        out=wb[:], in0=wraw[:],
                        scalar1=0.0, scalar2=0.5,
                        op0=mybir.AluOpType.is_ge,
                        op1=mybir.AluOpType.subtract,
                    )
                wbs.append(wb)

            # ---- x: 32 half-stripe loads [128, 1024] in (bhalf, it) order
            # ---- so the first oc pass can stream in arrival order ----
            xs = [[None, None] for _ in range(KT)]
            for bh in range(2):
                for it in range(KT):
                    xtile = persist.tile([128, B_SH // 2], BF16,
                                         name=f"xs{bh}_{it}")
                    eng = nc.sync if it % 2 == 0 else nc.scalar
                    eng.dma_start(
                        xtile[:],
                        xt[it * 128:(it + 1) * 128,
                           bh * (B_SH // 2):(bh + 1) * (B_SH // 2)])
                    xs[it][bh] = xtile

            # ---- persistent y (bf16, [o, b] layout) and stats ----
            y_all = persist.tile([128, OC * B_SH], BF16)   # [p, (oc, b)]
            sacc = persist.tile([128, 2 * OC], F32)        # [sy | sy2]

            # ---- main loop: oc outer, bc middle, it inner ----
            for oc in range(OC):
                pt = psum_pool.tile([128, BC * 512], F32, name=f"pt{oc}",
                                    tag="pt")
                for bc in range(BC):
                    for it in range(KT):
                        nc.tensor.matmul(
                            pt[:, bc * 512:bc * 512 + 512],
                            wbs[it][:, oc * 128:oc * 128 + 128],
                            xs[it][bc // 2][:, (bc % 2) * 512:
                                            (bc % 2) * 512 + 512],
                            start=(it == 0), stop=(it == KT - 1),
                        )
                # stats + copy out of PSUM; DVE does copy+sum(y), ACT does
                # square+sum(y^2), one [128,512] pass per bc each
                py = work_pool.tile([128, BC], F32, name=f"py{oc}",
                                    tag=f"py{oc % 2}")
                py2 = work_pool.tile([128, BC], F32, name=f"py2{oc}",
                                     tag=f"py2{oc % 2}")
                for bc in range(BC):
                    # copy PSUM->y_all (bf16) and row-sum in one DVE op
                    # (with accum_out, op1 is the reduction op)
                    nc.vector.tensor_scalar(
                        out=y_all[:, oc * B_SH + bc * 512:
                                  oc * B_SH + bc * 512 + 512],
                        in0=pt[:, bc * 512:bc * 512 + 512],
                        scalar1=1.0, scalar2=None,
                        op0=mybir.AluOpType.mult,
                        op1=mybir.AluOpType.add,
                        accum_out=py[:, bc:bc + 1],
                    )
                    y2scr = work_pool.tile([128, 512], BF16,
                                           name=f"y2_{oc}_{bc}",
                                           tag=f"y2_{bc % 2}")
                    nc.scalar.activation(
                        y2scr[:], pt[:, bc * 512:bc * 512 + 512],
                        mybir.ActivationFunctionType.Square,
                        accum_out=py2[:, bc:bc + 1],
                    )
                nc.vector.reduce_sum(out=sacc[:, oc:oc + 1], in_=py[:],
                                     axis=mybir.AxisListType.X)
                nc.vector.reduce_sum(out=sacc[:, OC + oc:OC + oc + 1],
                                     in_=py2[:], axis=mybir.AxisListType.X)

                if oc in WARM_OCS:
                    # dummy all-reduce: pays CC firmware init/warm-up cost
                    # off the critical path; nothing waits on its output
                    wsrc = work_pool.tile([1, 8], F32, name=f"wsrc{oc}",
                                          tag="wsrc")
                    nc.vector.memset(wsrc[:], 1.0)
                    wi = dram.tile([1, 8], F32, name=f"warm_i{oc}",
                                   tag=f"warm_i{oc}")
                    wo = dram.tile([1, 8], F32, name=f"warm_o{oc}",
                                   tag=f"warm_o{oc}")
                    nc.gpsimd.dma_start(wi[:], wsrc[:])
                    nc.gpsimd.collective_compute(
                        "AllReduce", mybir.AluOpType.add,
                        replica_groups=[list(range(N_CORES))],
                        ins=[wi.opt()], outs=[wo.opt()],
                    )

            # ---- global stats all-reduce ([128, 16] f32 = 8KB) ----
            cbi = dram.tile([128, 2 * OC], F32)
            cbo = dram.tile([128, 2 * OC], F32)
            nc.gpsimd.dma_start(cbi[:], sacc[:])
            nc.gpsimd.collective_compute(
                "AllReduce", mybir.AluOpType.add,
                replica_groups=[list(range(N_CORES))],
                ins=[cbi.opt()], outs=[cbo.opt()],
            )
            gs = persist.tile([128, 2 * OC], F32)
            nc.sync.dma_start(gs[:], cbo[:])

            # gamma/beta in [128, OC] layout
            gam = persist.tile([128, OC], F32)
            bet = persist.tile([128, OC], F32)
            nc.scalar.dma_start(gam[:], gamma[:, :])
            nc.scalar.dma_start(bet[:], beta[:, :])

            # ---- coefficients: a = gamma/sqrt(var+eps/4), c = beta - mean*a
            mean = persist.tile([128, OC], F32)
            var = persist.tile([128, OC], F32)
            inv = persist.tile([128, OC], F32)
            a_c = persist.tile([128, OC], F32)
            c_c = persist.tile([128, OC], F32)
            nc.vector.tensor_scalar_mul(mean[:], gs[:, 0:OC], 1.0 / B_TOT)
            # var = E[y^2] - mean^2 + eps/4 :
            #   m2 = mean*mean ; var = E[y^2]*(1/B) - m2 (+eps/4)
            nc.vector.tensor_scalar_mul(var[:], gs[:, OC:2 * OC], 1.0 / B_TOT)
            m2 = persist.tile([128, OC], F32)
            nc.vector.tensor_tensor(out=m2[:], in0=mean[:], in1=mean[:],
                                    op=mybir.AluOpType.mult)
            nc.vector.tensor_tensor(out=var[:], in0=var[:], in1=m2[:],
                                    op=mybir.AluOpType.subtract)
            nc.vector.tensor_scalar_add(var[:], var[:], BN_EPS / 4.0)
            nc.scalar.activation(inv[:], var[:],
                                 mybir.ActivationFunctionType.Sqrt)
            nc.vector.reciprocal(inv[:], inv[:])
            nc.vector.tensor_tensor(out=a_c[:], in0=gam[:], in1=inv[:],
                                    op=mybir.AluOpType.mult)
            ma = persist.tile([128, OC], F32)
            nc.vector.tensor_tensor(out=ma[:], in0=mean[:], in1=a_c[:],
                                    op=mybir.AluOpType.mult)
            nc.vector.tensor_tensor(out=c_c[:], in0=bet[:], in1=ma[:],
                                    op=mybir.AluOpType.subtract)

            # ---- normalize (per-partition scale+bias) and store ----
            # alternate DVE / ACT so both engines chew the tail in parallel
            for oc in range(OC):
                stg = stage_pool.tile([128, B_SH], BF16, name=f"stg{oc}",
                                      tag="stg")
                ysl = y_all[:, oc * B_SH:(oc + 1) * B_SH]
                if oc % 2 == 0:
                    nc.vector.tensor_scalar(
                        out=stg[:], in0=ysl,
                        scalar1=a_c[:, oc:oc + 1], scalar2=c_c[:, oc:oc + 1],
                        op0=mybir.AluOpType.mult,
                        op1=mybir.AluOpType.add,
                    )
                else:
                    nc.scalar.activation(
                        stg[:], ysl,
                        mybir.ActivationFunctionType.Identity,
                        bias=c_c[:, oc:oc + 1], scale=a_c[:, oc:oc + 1],
                    )
                eng = nc.sync if oc % 2 == 0 else nc.gpsimd
                eng.dma_start(out[oc * 128:(oc + 1) * 128, :], stg[:])

    nc.compile()
    return nc


_NC_CACHE = None


def kernel(x, weight, bias, gamma, beta):
    global _NC_CACHE
    if _NC_CACHE is None:
        _NC_CACHE = build_kernel()
    nc = _NC_CACHE

    import ml_dtypes
    bf16 = ml_dtypes.bfloat16

    x = np.asarray(x, dtype=np.float32)
    weight = np.asarray(weight, dtype=np.float32)
    # gamma/beta -> [128, OC] with partition p = o % 128, column oc
    gamma_t = np.ascontiguousarray(
        np.asarray(gamma, dtype=np.float32).reshape(OC, 128).T)
    beta_t = np.ascontiguousarray(
        np.asarray(beta, dtype=np.float32).reshape(OC, 128).T)

    wt = np.ascontiguousarray(weight.T).astype(bf16)
    in_maps = []
    for i in range(N_CORES):
        shard = x[i * B_SH:(i + 1) * B_SH]
        in_maps.append({
            "xt": np.ascontiguousarray(shard.T).astype(bf16),
            "wt": wt,
            "gamma": gamma_t,
            "beta": beta_t,
        })

    res = bass_utils.run_bass_kernel_spmd(
        nc, in_maps, core_ids=list(range(N_CORES)),
        trace=bool(int(os.environ.get("KERNEL_TRACE", "0"))),
    )
    kernel.last_results = res
    # device output is [o, b] bf16; transpose back and upcast
    return np.concatenate(
        [np.asarray(res.results[i]["out"]).T.astype(np.float32)
         for i in range(N_CORES)], axis=0)


# revision 4
# speedup vs baseline: 1.4771x; 1.4771x over previous
"""Trainium2 SPMD kernel for: y = BatchNorm1d(x @ sign(w).T + bias) * gamma + beta.

Sharding: data-parallel over the batch dim across 8 NeuronCores; the
weight is replicated.  BatchNorm batch statistics are produced with an
on-device AllReduce of per-shard (sum_y, sum_y2).

Layout: weight-stationary matmul producing y in [o, b] layout (output
features on partitions).  This makes the batch reductions free-dim
reductions (fused into the PSUM->SBUF copy via accum_out on DVE and the
Square pass on ACT), removes all stats matmuls from the tensor engine,
and turns the BN normalize into a single per-partition scale+bias op.

Math notes:
  - The linear bias cancels inside BatchNorm (y - mean), so it is never
    applied on device.
  - sign(w) in {-1,+1} is realized as +-0.5 via integer ops on the bf16
    bit pattern: (w & 0x8000) | 0x3F00.  The global scale of 0.5 cancels
    in BatchNorm except in the epsilon, which is compensated with eps/4.
    (A float is_ge on bf16 input hits a slow DVE microcode path; the
    bitwise form runs at full rate.)
  - Matmul runs in bf16 (weights +-0.5 exact; x rounding ~2e-3 rel err).
    x and w are cast to bf16 on the host (same numerics as a casting
    DMA, half the HBM traffic).  The output is written bf16 and upcast
    on the host (the values are bf16-rounded before the store either
    way, so this is numerically identical to a casting store DMA).
"""

import os
import sys

sys.path.insert(0, "/opt/trn_rl_repo")

import numpy as np

import concourse.bacc as bacc
import concourse.mybir as mybir
import concourse.tile as tile
from concourse import bass_utils

N_CORES = 8
B_TOT = 16384
D_IN = 2048
D_OUT = 1024
B_SH = B_TOT // N_CORES          # 2048 rows per core
KT = D_IN // 128                 # 16 contraction stripes
OC = D_OUT // 128                # 8 output chunks (partition dim of y)
BC = B_SH // 512                 # 4 batch chunks of 512 per core
BN_EPS = 1e-5

F32 = mybir.dt.float32
BF16 = mybir.dt.bfloat16
U16 = mybir.dt.uint16

# oc passes after which a dummy warm-up all-reduce fires (keeps the CC
# firmware initialized/warm so the real stats AR at the end is cheap)
WARM_OCS = tuple(
    int(s) for s in os.environ.get("KERNEL_WARM_OCS", "0,4").split(",") if s != ""
)


def build_kernel():
    nc = bacc.Bacc("TRN2", target_bir_lowering=False, debug=False,
                   num_devices=N_CORES)

    # x^T per shard: [k, b], contiguous 4KB rows
    xt = nc.dram_tensor("xt", [D_IN, B_SH], BF16, kind="ExternalInput")
    # w^T: [k, o], contiguous 2KB rows
    wt = nc.dram_tensor("wt", [D_IN, D_OUT], BF16, kind="ExternalInput")
    # gamma/beta pre-laid-out as [128, OC] (partition p = o % 128, col oc)
    gamma = nc.dram_tensor("gamma", [128, OC], F32, kind="ExternalInput")
    beta = nc.dram_tensor("beta", [128, OC], F32, kind="ExternalInput")
    # output in device layout [oc*128 + p, b]; host transposes back
    out = nc.dram_tensor("out", [D_OUT, B_SH], BF16, kind="ExternalOutput")

    with tile.TileContext(nc) as tc:
        with tc.tile_pool(name="persist", bufs=1) as persist, \
             tc.tile_pool(name="work", bufs=2) as work_pool, \
             tc.tile_pool(name="stage", bufs=3) as stage_pool, \
             tc.tile_pool(name="psum", bufs=2, space="PSUM") as psum_pool, \
             tc.tile_pool(name="dram", bufs=1, space="DRAM") as dram:

            # ---- per-stripe loads: w (scalar ring) + x (sync ring),
            # ---- interleaved so stripe `it` lands early; binarize w to
            # ---- {-0.5,+0.5} with integer bit ops on DVE (full rate)
            wbs = []
            xs = []
            for it in range(KT):
                wraw = work_pool.tile([128, D_OUT], BF16, name=f"wraw{it}",
                                      tag=f"wraw{it % 4}")
                nc.scalar.dma_start(wraw[:], wt[it * 128:(it + 1) * 128, :])
                wb = persist.tile([128, D_OUT], BF16, name=f"wb{it}")
                # (w & 0x8000) | 0x3F00  ->  +-0.5 in bf16
                nc.vector.tensor_scalar(
                    out=wb.bitcast(U16)[:], in0=wraw.bitcast(U16)[:],
                    scalar1=0x8000, scalar2=0x3F00,
                    op0=mybir.AluOpType.bitwise_and,
                    op1=mybir.AluOpType.bitwise_or,
                )
                wbs.append(wb)

                xtile = persist.tile([128, B_SH], BF16, name=f"xs{it}")
                nc.sync.dma_start(xtile[:], xt[it * 128:(it + 1) * 128, :])
                xs.append(xtile)

            # ---- persistent y (bf16, [o, b] layout) and stats ----
            y_all = persist.tile([128, OC * B_SH], BF16)   # [p, (oc, b)]
            sacc = persist.tile([128, 2 * OC], F32)        # [sy | sy2]

            # ---- main loop: oc outer, it middle, bc inner (4 MMs share a
            # ---- stationary weight chunk; x streamed in stripe order) ----
            for oc in range(OC):
                pt = psum_pool.tile([128, BC * 512], F32, name=f"pt{oc}",
                                    tag="pt")
                for it in range(KT):
                    for bc in range(BC):
                        nc.tensor.matmul(
                            pt[:, bc * 512:bc * 512 + 512],
                            wbs[it][:, oc * 128:oc * 128 + 128],
                            xs[it][:, bc * 512:bc * 512 + 512],
                            start=(it == 0), stop=(it == KT - 1),
                        )
                # stats + copy out of PSUM; DVE does copy+sum(y), ACT does
                # square+sum(y^2), one [128,512] pass per bc each
                py = work_pool.tile([128, BC], F32, name=f"py{oc}",
                                    tag=f"py{oc % 2}")
                py2 = work_pool.tile([128, BC], F32, name=f"py2{oc}",
                                     tag=f"py2{oc % 2}")
                for bc in range(BC):
                    # copy PSUM->y_all (bf16) and row-sum in one DVE op
                    # (with accum_out, op1 is the reduction op)
                    nc.vector.tensor_scalar(
                        out=y_all[:, oc * B_SH + bc * 512:
                                  oc * B_SH + bc * 512 + 512],
                        in0=pt[:, bc * 512:bc * 512 + 512],
                        scalar1=1.0, scalar2=None,
                        op0=mybir.AluOpType.mult,
                        op1=mybir.AluOpType.add,
                        accum_out=py[:, bc:bc + 1],
                    )
                    y2scr = work_pool.tile([128, 512], BF16,
                                           name=f"y2_{oc}_{bc}",
                                           tag=f"y2_{bc % 2}")
                    nc.scalar.activation(
                        y2scr[:], pt[:, bc * 512:bc * 512 + 512],
                        mybir.ActivationFunctionType.Square,
                        accum_out=py2[:, bc:bc + 1],
                    )
                nc.vector.reduce_sum(out=sacc[:, oc:oc + 1], in_=py[:],
                                     axis=mybir.AxisListType.X)
                nc.vector.reduce_sum(out=sacc[:, OC + oc:OC + oc + 1],
                                     in_=py2[:], axis=mybir.AxisListType.X)

                if oc in WARM_OCS:
                    # dummy all-reduce: pays CC firmware init/warm-up cost
                    # off the critical path; nothing waits on its output
                    wsrc = work_pool.tile([1, 8], F32, name=f"wsrc{oc}",
                                          tag="wsrc")
                    nc.vector.memset(wsrc[:], 1.0)
                    wi = dram.tile([1, 8], F32, name=f"warm_i{oc}",
                                   tag=f"warm_i{oc}")
                    wo = dram.tile([1, 8], F32, name=f"warm_o{oc}",
                                   tag=f"warm_o{oc}")
                    nc.gpsimd.dma_start(wi[:], wsrc[:])
                    nc.gpsimd.collective_compute(
                        "AllReduce", mybir.AluOpType.add,
                        replica_groups=[list(range(N_CORES))],
                        ins=[wi.opt()], outs=[wo.opt()],
                    )

            # ---- global stats all-reduce ([128, 16] f32 = 8KB) ----
            cbi = dram.tile([128, 2 * OC], F32)
            cbo = dram.tile([128, 2 * OC], F32)
            nc.sync.dma_start(cbi[:], sacc[:])
            nc.gpsimd.collective_compute(
                "AllReduce", mybir.AluOpType.add,
                replica_groups=[list(range(N_CORES))],
                ins=[cbi.opt()], outs=[cbo.opt()],
            )
            gs = persist.tile([128, 2 * OC], F32)
            nc.sync.dma_start(gs[:], cbo[:])

            # gamma/beta in [128, OC] layout
            gam = persist.tile([128, OC], F32)
            bet = persist.tile([128, OC], F32)
            nc.scalar.dma_start(gam[:], gamma[:, :])
            nc.scalar.dma_start(bet[:], beta[:, :])

            # ---- coefficients: a = gamma/sqrt(var+eps/4), c = beta - mean*a
            mean = persist.tile([128, OC], F32)
            var = persist.tile([128, OC], F32)
            inv = persist.tile([128, OC], F32)
            a_c = persist.tile([128, OC], F32)
            c_c = persist.tile([128, OC], F32)
            nc.vector.tensor_scalar_mul(mean[:], gs[:, 0:OC], 1.0 / B_TOT)
            nc.vector.tensor_scalar_mul(var[:], gs[:, OC:2 * OC], 1.0 / B_TOT)
            m2 = persist.tile([128, OC], F32)
            nc.vector.tensor_tensor(out=m2[:], in0=mean[:], in1=mean[:],
                                    op=mybir.AluOpType.mult)
            nc.vector.tensor_tensor(out=var[:], in0=var[:], in1=m2[:],
                                    op=mybir.AluOpType.subtract)
            nc.vector.tensor_scalar_add(var[:], var[:], BN_EPS / 4.0)
            nc.scalar.activation(inv[:], var[:],
                                 mybir.ActivationFunctionType.Sqrt)
            nc.vector.reciprocal(inv[:], inv[:])
            nc.vector.tensor_tensor(out=a_c[:], in0=gam[:], in1=inv[:],
                                    op=mybir.AluOpType.mult)
            ma = persist.tile([128, OC], F32)
            nc.vector.tensor_tensor(out=ma[:], in0=mean[:], in1=a_c[:],
                                    op=mybir.AluOpType.mult)
            nc.vector.tensor_tensor(out=c_c[:], in0=bet[:], in1=ma[:],
                                    op=mybir.AluOpType.subtract)

            # ---- normalize (per-partition scale+bias) and store ----
            # alternate DVE / ACT so both engines chew the tail in parallel
            for oc in range(OC):
                stg = stage_pool.tile([128, B_SH], BF16, name=f"stg{oc}",
                                      tag="stg")
                ysl = y_all[:, oc * B_SH:(oc + 1) * B_SH]
                if oc % 2 == 0:
                    nc.vector.tensor_scalar(
                        out=stg[:], in0=ysl,
                        scalar1=a_c[:, oc:oc + 1], scalar2=c_c[:, oc:oc + 1],
                        op0=mybir.AluOpType.mult,
                        op1=mybir.AluOpType.add,
                    )
                else:
                    nc.scalar.activation(
                        stg[:], ysl,
                        mybir.ActivationFunctionType.Identity,
                        bias=c_c[:, oc:oc + 1], scale=a_c[:, oc:oc + 1],
                    )
                eng = nc.sync if oc % 2 == 0 else nc.scalar
                eng.dma_start(out[oc * 128:(oc + 1) * 128, :], stg[:])

    nc.compile()
    return nc


_NC_CACHE = None


def kernel(x, weight, bias, gamma, beta):
    global _NC_CACHE
    if _NC_CACHE is None:
        _NC_CACHE = build_kernel()
    nc = _NC_CACHE

    import ml_dtypes
    bf16 = ml_dtypes.bfloat16

    x = np.asarray(x, dtype=np.float32)
    weight = np.asarray(weight, dtype=np.float32)
    # gamma/beta -> [128, OC] with partition p = o % 128, column oc
    gamma_t = np.ascontiguousarray(
        np.asarray(gamma, dtype=np.float32).reshape(OC, 128).T)
    beta_t = np.ascontiguousarray(
        np.asarray(beta, dtype=np.float32).reshape(OC, 128).T)

    wt = np.ascontiguousarray(weight.T).astype(bf16)
    in_maps = []
    for i in range(N_CORES):
        shard = x[i * B_SH:(i + 1) * B_SH]
        in_maps.append({
            "xt": np.ascontiguousarray(shard.T).astype(bf16),
            "wt": wt,
            "gamma": gamma_t,
            "beta": beta_t,
        })

    res = bass_utils.run_bass_kernel_spmd(
        nc, in_maps, core_ids=list(range(N_CORES)),
        trace=bool(int(os.environ.get("KERNEL_TRACE", "0"))),
    )
    kernel.last_results = res
    # device output is [o, b] bf16; transpose back and upcast
    return np.concatenate(
        [np.asarray(res.results[i]["out"]).T.astype(np.float32)
         for i in range(N_CORES)], axis=0)


# revision 6
# speedup vs baseline: 1.6394x; 1.1099x over previous
"""Trainium2 SPMD kernel for: y = BatchNorm1d(x @ sign(w).T + bias) * gamma + beta.

Sharding: data-parallel over the batch dim across 8 NeuronCores; the
weight is replicated.  BatchNorm batch statistics are produced with
on-device AllReduces of per-shard (sum_y, sum_y2).

Layout: weight-stationary matmul producing y in [o, b] layout (output
features on partitions).  This makes the batch reductions free-dim
reductions (fused into the PSUM->SBUF copy via accum_out on DVE and a
Square pass on ACT), removes all stats matmuls from the tensor engine,
and turns the BN normalize into a single per-partition scale+bias op.

Pipelining: the stats AllReduce is split in two.  The first half
(output chunks 0..3) reduces, all-reduces, and normalizes *during* the
second half's matmuls; only the second AllReduce and the last four
normalize+store chunks sit after the final matmul.

Math notes:
  - The linear bias cancels inside BatchNorm (y - mean), so it is never
    applied on device.
  - sign(w) in {-1,+1} is realized as +-0.5 via integer ops on the bf16
    bit pattern: (w & 0x8000) | 0x3F00.  The global scale of 0.5 cancels
    in BatchNorm except in the epsilon, which is compensated with eps/4.
    (A float is_ge on bf16 input hits a slow DVE microcode path; the
    bitwise form runs at full rate.)
  - Matmul runs in bf16 (weights +-0.5 exact; x rounding ~2e-3 rel err).
    x and w are cast to bf16 on the host (same numerics as a casting
    DMA, half the HBM traffic).  The output is written bf16 and upcast
    on the host (the values are bf16-rounded before the store either
    way, so this is numerically identical to a casting store DMA).
"""

import os
import sys

sys.path.insert(0, "/opt/trn_rl_repo")

import numpy as np

import concourse.bacc as bacc
import concourse.mybir as mybir
import concourse.tile as tile
from concourse import bass_utils

N_CORES = 8
B_TOT = 16384
D_IN = 2048
D_OUT = 1024
B_SH = B_TOT // N_CORES          # 2048 rows per core
KT = D_IN // 128                 # 16 contraction stripes
OC = D_OUT // 128                # 8 output chunks (partition dim of y)
NB = int(os.environ.get("KERNEL_MM_N", "512"))   # moving free dim per MM
BC = B_SH // NB                  # batch chunks per core
BN_EPS = 1e-5

F32 = mybir.dt.float32
BF16 = mybir.dt.bfloat16
U16 = mybir.dt.uint16

# oc passes after which a dummy warm-up all-reduce fires (pays the CC
# firmware init cost off the critical path)
WARM_OCS = tuple(
    int(s) for s in os.environ.get("KERNEL_WARM_OCS", "0").split(",") if s != ""
)


def build_kernel():
    nc = bacc.Bacc("TRN2", target_bir_lowering=False, debug=False,
                   num_devices=N_CORES)

    # x^T per shard: [k, b], contiguous 4KB rows
    xt = nc.dram_tensor("xt", [D_IN, B_SH], BF16, kind="ExternalInput")
    # w^T: [k, o], contiguous 2KB rows
    wt = nc.dram_tensor("wt", [D_IN, D_OUT], BF16, kind="ExternalInput")
    # gamma/beta pre-laid-out as [128, OC] (partition p = o % 128, col oc)
    gamma = nc.dram_tensor("gamma", [128, OC], F32, kind="ExternalInput")
    beta = nc.dram_tensor("beta", [128, OC], F32, kind="ExternalInput")
    # output in device layout [oc*128 + p, b]; host transposes back
    out = nc.dram_tensor("out", [D_OUT, B_SH], BF16, kind="ExternalOutput")

    with tile.TileContext(nc) as tc:
        with tc.tile_pool(name="persist", bufs=1) as persist, \
             tc.tile_pool(name="work", bufs=2) as work_pool, \
             tc.tile_pool(name="stage", bufs=3) as stage_pool, \
             tc.tile_pool(name="psum", bufs=2, space="PSUM") as psum_pool, \
             tc.tile_pool(name="dram", bufs=1, space="DRAM") as dram:

            # ---- per-stripe loads: w (scalar ring) + x (sync ring),
            # ---- interleaved so stripe `it` lands early; binarize w to
            # ---- {-0.5,+0.5} with integer bit ops on DVE (full rate)
            wbs = []
            xs = []
            for it in range(KT):
                wraw = work_pool.tile([128, D_OUT], BF16, name=f"wraw{it}",
                                      tag=f"wraw{it % 4}")
                nc.scalar.dma_start(wraw[:], wt[it * 128:(it + 1) * 128, :])
                wb = persist.tile([128, D_OUT], BF16, name=f"wb{it}")
                # (w & 0x8000) | 0x3F00  ->  +-0.5 in bf16
                nc.vector.tensor_scalar(
                    out=wb.bitcast(U16)[:], in0=wraw.bitcast(U16)[:],
                    scalar1=0x8000, scalar2=0x3F00,
                    op0=mybir.AluOpType.bitwise_and,
                    op1=mybir.AluOpType.bitwise_or,
                )
                wbs.append(wb)

                xtile = persist.tile([128, B_SH], BF16, name=f"xs{it}")
                nc.sync.dma_start(xtile[:], xt[it * 128:(it + 1) * 128, :])
                xs.append(xtile)

            # ---- persistent y (bf16, [o, b] layout) and per-half stats ----
            y_all = persist.tile([128, OC * B_SH], BF16)   # [p, (oc, b)]
            # halves: cols [sy(4) | sy2(4)]
            sacc = [persist.tile([128, 8], F32, name=f"sacc{h}")
                    for h in range(2)]

            # gamma/beta in [128, OC] layout (loaded up front; tiny)
            gam = persist.tile([128, OC], F32)
            bet = persist.tile([128, OC], F32)
            nc.scalar.dma_start(gam[:], gamma[:, :])
            nc.scalar.dma_start(bet[:], beta[:, :])

            # coefficient tiles (written per half, read per oc chunk)
            a_c = persist.tile([128, OC], F32)
            c_c = persist.tile([128, OC], F32)
            mean = persist.tile([128, OC], F32)
            var = persist.tile([128, OC], F32)
            inv = persist.tile([128, OC], F32)
            m2 = persist.tile([128, OC], F32)
            ma = persist.tile([128, OC], F32)

            cbis = [dram.tile([128, 8], F32, name=f"cbi{h}", tag=f"cbi{h}")
                    for h in range(2)]
            cbos = [dram.tile([128, 8], F32, name=f"cbo{h}", tag=f"cbo{h}")
                    for h in range(2)]
            gss = [persist.tile([128, 8], F32, name=f"gs{h}")
                   for h in range(2)]

            def do_half_ar(h):
                """DMA stats of half h out, all-reduce, read back."""
                nc.sync.dma_start(cbis[h][:], sacc[h][:])
                nc.gpsimd.collective_compute(
                    "AllReduce", mybir.AluOpType.add,
                    replica_groups=[list(range(N_CORES))],
                    ins=[cbis[h].opt()], outs=[cbos[h].opt()],
                )
                nc.sync.dma_start(gss[h][:], cbos[h][:])

            def do_half_coef(h):
                """a = gamma/sqrt(var+eps/4), c = beta - mean*a for half h."""
                lo, hi = 4 * h, 4 * h + 4
                gs = gss[h]
                nc.vector.tensor_scalar_mul(mean[:, lo:hi], gs[:, 0:4],
                                            1.0 / B_TOT)
                nc.vector.tensor_scalar_mul(var[:, lo:hi], gs[:, 4:8],
                                            1.0 / B_TOT)
                nc.vector.tensor_tensor(out=m2[:, lo:hi], in0=mean[:, lo:hi],
                                        in1=mean[:, lo:hi],
                                        op=mybir.AluOpType.mult)
                nc.vector.tensor_tensor(out=var[:, lo:hi], in0=var[:, lo:hi],
                                        in1=m2[:, lo:hi],
                                        op=mybir.AluOpType.subtract)
                nc.vector.tensor_scalar_add(var[:, lo:hi], var[:, lo:hi],
                                            BN_EPS / 4.0)
                nc.scalar.activation(inv[:, lo:hi], var[:, lo:hi],
                                     mybir.ActivationFunctionType.Sqrt)
                nc.vector.reciprocal(inv[:, lo:hi], inv[:, lo:hi])
                nc.vector.tensor_tensor(out=a_c[:, lo:hi], in0=gam[:, lo:hi],
                                        in1=inv[:, lo:hi],
                                        op=mybir.AluOpType.mult)
                nc.vector.tensor_tensor(out=ma[:, lo:hi], in0=mean[:, lo:hi],
                                        in1=a_c[:, lo:hi],
                                        op=mybir.AluOpType.mult)
                nc.vector.tensor_tensor(out=c_c[:, lo:hi], in0=bet[:, lo:hi],
                                        in1=ma[:, lo:hi],
                                        op=mybir.AluOpType.subtract)

            def do_norm_store(oc, on_vector):
                """Normalize y chunk oc with per-partition a,c and store."""
                stg = stage_pool.tile([128, B_SH], BF16, name=f"stg{oc}",
                                      tag="stg")
                ysl = y_all[:, oc * B_SH:(oc + 1) * B_SH]
                if on_vector:
                    nc.vector.tensor_scalar(
                        out=stg[:], in0=ysl,
                        scalar1=a_c[:, oc:oc + 1], scalar2=c_c[:, oc:oc + 1],
                        op0=mybir.AluOpType.mult,
                        op1=mybir.AluOpType.add,
                    )
                else:
                    nc.scalar.activation(
                        stg[:], ysl,
                        mybir.ActivationFunctionType.Identity,
                        bias=c_c[:, oc:oc + 1], scale=a_c[:, oc:oc + 1],
                    )
                eng = nc.sync if oc % 2 == 0 else nc.scalar
                eng.dma_start(out[oc * 128:(oc + 1) * 128, :], stg[:])

            # ---- main loop: oc outer, it middle, bc inner ----
            for oc in range(OC):
                h = oc // 4
                pt = psum_pool.tile([128, B_SH], F32, name=f"pt{oc}",
                                    tag="pt")
                for it in range(KT):
                    for bc in range(BC):
                        nc.tensor.matmul(
                            pt[:, bc * NB:bc * NB + NB],
                            wbs[it][:, oc * 128:oc * 128 + 128],
                            xs[it][:, bc * NB:bc * NB + NB],
                            start=(it == 0), stop=(it == KT - 1),
                        )
                # stats + copy out of PSUM: DVE copies+sums y (4x [128,512]);
                # ACT squares+sums from the bf16 copy (keeps PSUM single-
                # reader and lets the two engines pipeline per chunk)
                py = work_pool.tile([128, 4], F32, name=f"py{oc}",
                                    tag=f"py{oc % 2}")
                py2 = work_pool.tile([128, 4], F32, name=f"py2{oc}",
                                     tag=f"py2{oc % 2}")
                for q in range(4):
                    nc.vector.tensor_scalar(
                        out=y_all[:, oc * B_SH + q * 512:
                                  oc * B_SH + q * 512 + 512],
                        in0=pt[:, q * 512:q * 512 + 512],
                        scalar1=1.0, scalar2=None,
                        op0=mybir.AluOpType.mult,
                        op1=mybir.AluOpType.add,
                        accum_out=py[:, q:q + 1],
                    )
                for q in range(4):
                    y2scr = work_pool.tile([128, 512], BF16,
                                           name=f"y2_{oc}_{q}",
                                           tag=f"y2_{q % 2}")
                    nc.scalar.activation(
                        y2scr[:],
                        y_all[:, oc * B_SH + q * 512:oc * B_SH + q * 512 + 512],
                        mybir.ActivationFunctionType.Square,
                        accum_out=py2[:, q:q + 1],
                    )
                oc4 = oc % 4
                nc.vector.reduce_sum(out=sacc[h][:, oc4:oc4 + 1], in_=py[:],
                                     axis=mybir.AxisListType.X)
                nc.vector.reduce_sum(out=sacc[h][:, 4 + oc4:5 + oc4],
                                     in_=py2[:], axis=mybir.AxisListType.X)

                if oc in WARM_OCS:
                    # dummy all-reduce: pays CC firmware init cost early;
                    # nothing waits on its output
                    wsrc = work_pool.tile([1, 8], F32, name=f"wsrc{oc}",
                                          tag="wsrc")
                    nc.vector.memset(wsrc[:], 1.0)
                    wi = dram.tile([1, 8], F32, name=f"warm_i{oc}",
                                   tag=f"warm_i{oc}")
                    wo = dram.tile([1, 8], F32, name=f"warm_o{oc}",
                                   tag=f"warm_o{oc}")
                    nc.gpsimd.dma_start(wi[:], wsrc[:])
                    nc.gpsimd.collective_compute(
                        "AllReduce", mybir.AluOpType.add,
                        replica_groups=[list(range(N_CORES))],
                        ins=[wi.opt()], outs=[wo.opt()],
                    )

                if oc == 3:
                    # first-half stats AR overlaps the oc4..7 matmuls
                    do_half_ar(0)
                if oc == 5:
                    do_half_coef(0)
                if oc == 6:
                    # normalize+store oc0..3 while oc7 computes
                    do_norm_store(0, on_vector=True)
                    do_norm_store(1, on_vector=False)
                    do_norm_store(2, on_vector=True)
                    do_norm_store(3, on_vector=True)

            # ---- second-half stats AR + tail normalize ----
            do_half_ar(1)
            do_half_coef(1)
            do_norm_store(4, on_vector=True)
            do_norm_store(5, on_vector=False)
            do_norm_store(6, on_vector=True)
            do_norm_store(7, on_vector=True)

    nc.compile()
    return nc


_NC_CACHE = None


def kernel(x, weight, bias, gamma, beta):
    global _NC_CACHE
    if _NC_CACHE is None:
        _NC_CACHE = build_kernel()
    nc = _NC_CACHE

    import ml_dtypes
    bf16 = ml_dtypes.bfloat16

    x = np.asarray(x, dtype=np.float32)
    weight = np.asarray(weight, dtype=np.float32)
    # gamma/beta -> [128, OC] with partition p = o % 128, column oc
    gamma_t = np.ascontiguousarray(
        np.asarray(gamma, dtype=np.float32).reshape(OC, 128).T)
    beta_t = np.ascontiguousarray(
        np.asarray(beta, dtype=np.float32).reshape(OC, 128).T)

    wt = np.ascontiguousarray(weight.T).astype(bf16)
    in_maps = []
    for i in range(N_CORES):
        shard = x[i * B_SH:(i + 1) * B_SH]
        in_maps.append({
            "xt": np.ascontiguousarray(shard.T).astype(bf16),
            "wt": wt,
            "gamma": gamma_t,
            "beta": beta_t,
        })

    res = bass_utils.run_bass_kernel_spmd(
        nc, in_maps, core_ids=list(range(N_CORES)),
        trace=bool(int(os.environ.get("KERNEL_TRACE", "0"))),
    )
    kernel.last_results = res
    # device output is [o, b] bf16; transpose back and upcast
    return np.concatenate(
        [np.asarray(res.results[i]["out"]).T.astype(np.float32)
         for i in range(N_CORES)], axis=0)
